# revision 1
# baseline (speedup 1.0000x reference)
"""Trainium2 Bass kernel for nn_MultiHeadAttention (B=4, S=2048, D=1024, H=16, DH=64).

Sharding: 8 cores = 4 batches x 2 query-halves. Each core computes, for its
(batch b, query half): Q/K/V projections, masked softmax attention over the
full key length, and the output projection, entirely on-device.

Device-side layout is fully transposed (feature-major) so every matmul has its
contraction on the partition dim:
  QT = Wq^T xqT / 8         [HDH, Sq]   (1/8 score scale + bq folded in)
  KT = Wk^T xkT             [HDH, S]    (spilled to DRAM, per-pair reload)
  V  = (Wv^T xvT)^T         [S, HDH]    stored interleaved per head as
                                        [s, h, 65] with a ones column, so the
                                        PV matmul's PSUM row 64 accumulates
                                        l = sum_k exp(scoresT) for free
  scoresT_h = KT_h^T QT_h   [S, Sq]     (per head; 2 heads packed in PE rows)
  expT = exp(scoresT + mask_bias[k])    (no row-max: |scores| <= ~4)
  outT_h = V_h^T expT / l   [DH, Sq]
  yT = Wo^T outT + bo'      [D, Sq]
Key-padding mask enters as a per-partition bias (0 / -40) on the Exp
activation. bk is dropped (softmax-invariant); bv,bo fold into bo' = bv@Wo+bo
host-side (exact, since softmax rows sum to 1).
"""

import os
import sys
import numpy as np

if "/opt/trn_rl_repo" not in sys.path:
    sys.path.insert(0, "/opt/trn_rl_repo")

import concourse.bass as bass
import concourse.mybir as mybir
import concourse.tile as tile
from concourse import bacc
from concourse.bass_utils import run_bass_kernel_spmd

B, S, D = 4, 2048, 1024
H, DH = 16, 64
HDH = H * DH                      # 1024
SQ = S // 2                       # 1024 queries per core
P = 128
DC = D // P                       # 8 contraction chunks
NJ = 8                            # head pairs (2 heads x 64 rows = 128)
KC = S // P                       # 16 key chunks
SC = S // P                       # 16 s chunks for V
VW = DH + 1                       # 65: V columns per head + ones column
F32 = mybir.dt.float32
F32R = mybir.dt.float32r
BF16 = mybir.dt.bfloat16
MASK_NEG = -40.0

_CACHE = {}


def build_bass(kc_lim=KC):
    nc = bacc.Bacc("TRN2", target_bir_lowering=False, debug=False)
    klen = kc_lim * P                # keys actually attended (rest fully masked)
    kpad = ((klen + 511) // 512) * 512   # KT buffers padded to 512-col blocks

    xqT = nc.dram_tensor("xqT", [D, SQ], F32R, kind="ExternalInput").ap()
    xkT = nc.dram_tensor("xkT", [D, S], F32R, kind="ExternalInput").ap()
    xvT = nc.dram_tensor("xvT", [D, S], F32R, kind="ExternalInput").ap()
    wq = nc.dram_tensor("wq", [D, HDH], F32R, kind="ExternalInput").ap()
    wk = nc.dram_tensor("wk", [D, HDH], F32R, kind="ExternalInput").ap()
    wv = nc.dram_tensor("wv", [D, HDH], F32R, kind="ExternalInput").ap()
    wo = nc.dram_tensor("wo", [HDH, D], F32R, kind="ExternalInput").ap()
    bq8 = nc.dram_tensor("bq8", [P, NJ], F32, kind="ExternalInput").ap()
    bo2 = nc.dram_tensor("bo2", [P, DC], F32, kind="ExternalInput").ap()
    maskb = nc.dram_tensor("maskb", [P, KC], F32, kind="ExternalInput").ap()
    yT = nc.dram_tensor("yT", [D, SQ], F32, kind="ExternalOutput").ap()

    Exp = mybir.ActivationFunctionType.Exp
    AOp = mybir.AluOpType

    with tile.TileContext(nc) as tc:
        with (
            tc.tile_pool(name="const", bufs=1) as cpool,
            tc.tile_pool(name="vres", bufs=1) as vpool,
            tc.tile_pool(name="ktdram", bufs=1, space="DRAM") as ktd,
            tc.tile_pool(name="rdram", bufs=2, space="DRAM") as rdp,
        ):
            maskb_sb = cpool.tile([P, KC], F32)
            nc.sync.dma_start(out=maskb_sb, in_=maskb)
            bq8_sb = cpool.tile([P, NJ], F32)
            nc.sync.dma_start(out=bq8_sb, in_=bq8)
            bo2_sb = cpool.tile([P, DC], F32)
            nc.sync.dma_start(out=bo2_sb, in_=bo2)

            # V interleaved per head: [p, sc, h, 65]; col 64 of each head = 1.0
            v_sb = vpool.tile([P, SC, H, VW], F32R)
            kt_dram = [
                ktd.tile([P, kpad], F32R, tag=f"ktd{j}", name=f"ktd{j}")
                for j in range(NJ)
            ]

            # ---- phase V: V = (Wv^T xvT)^T, all pairs --------------------
            # x loaded in column blocks so compute starts after block 0 and
            # freed blocks let the next phase's loads overlap this compute.
            with (
                tc.tile_pool(name="xv", bufs=1) as xvp,
                tc.tile_pool(name="wvp", bufs=1) as wvp,
                tc.tile_pool(name="pv", bufs=2, space="PSUM") as pvp,
            ):
                wv_sb = wvp.tile([P, DC, HDH], F32R)
                wv_ch = wv.rearrange("(c p) n -> p c n", p=P)
                xv_cb = []
                xv_ch = xvT.rearrange("(c p) s -> p c s", p=P)
                for cb in range(4):
                    t = xvp.tile([P, DC, 512], F32R, tag=f"xv{cb}",
                                 name=f"xv{cb}")
                    xv_cb.append(t)
                nc.sync.dma_start(
                    out=xv_cb[0], in_=xv_ch[:, :, 0:512]
                )
                for kc in range(DC):
                    nc.scalar.dma_start(out=wv_sb[:, kc, :], in_=wv_ch[:, kc, :])
                for cb in range(1, 4):
                    nc.sync.dma_start(
                        out=xv_cb[cb], in_=xv_ch[:, :, cb * 512:(cb + 1) * 512]
                    )
                for sc in range(kc_lim):
                    cb, scl = sc // 4, sc % 4
                    ps = pvp.tile([P, HDH], F32, tag="pv")
                    for kc in range(DC):
                        lhsT = xv_cb[cb][:, kc, scl * P:(scl + 1) * P]
                        for nh in range(2):
                            nc.tensor.matmul(
                                ps[:, nh * 512:(nh + 1) * 512],
                                lhsT,
                                wv_sb[:, kc, nh * 512:(nh + 1) * 512],
                                start=(kc == 0),
                                stop=(kc == DC - 1),
                            )
                    # strided copy into the interleaved layout
                    nc.vector.tensor_copy(
                        v_sb[:, sc, :, 0:DH],
                        ps.rearrange("p (h d) -> p h d", d=DH),
                    )
                    nc.vector.tensor_scalar(
                        v_sb[:, sc, :, DH:VW], v_sb[:, sc, :, 0:1],
                        0.0, 1.0, AOp.mult, AOp.add,
                    )

            # ---- phase K: KT -> DRAM, all pairs --------------------------
            with (
                tc.tile_pool(name="xk", bufs=1) as xkp,
                tc.tile_pool(name="wkp", bufs=1) as wkp,
                tc.tile_pool(name="ktst", bufs=3) as ktstp,
                tc.tile_pool(name="pk", bufs=2, space="PSUM") as pkp,
            ):
                xk_cb = []
                xk_ch = xkT.rearrange("(c p) s -> p c s", p=P)
                for cb in range(4):
                    t = xkp.tile([P, DC, 512], F32R, tag=f"xk{cb}",
                                 name=f"xk{cb}")
                    eng = nc.sync if cb % 2 == 0 else nc.scalar
                    eng.dma_start(
                        out=t, in_=xk_ch[:, :, cb * 512:(cb + 1) * 512]
                    )
                    xk_cb.append(t)
                wk_sb = wkp.tile([P, DC, HDH], F32R)
                wk_ch = wk.rearrange("(c p) n -> p c n", p=P)
                for kc in range(DC):
                    eng = nc.scalar if kc % 2 == 0 else nc.sync
                    eng.dma_start(out=wk_sb[:, kc, :], in_=wk_ch[:, kc, :])
                n_kb = kpad // 512   # 512-col blocks of KT kept
                for half in range(2):
                    blks = [b for b in (2 * half, 2 * half + 1) if b < n_kb]
                    if not blks:
                        continue
                    w_cols = 512 * len(blks)
                    for j in range(NJ):
                        ps = pkp.tile([P, SQ], F32, tag="pk")
                        for kc in range(DC):
                            lhsT = wk_sb[:, kc, j * P:(j + 1) * P]
                            for bi, b in enumerate(blks):
                                nc.tensor.matmul(
                                    ps[:, bi * 512:(bi + 1) * 512],
                                    lhsT,
                                    xk_cb[b][:, kc, :],
                                    start=(kc == 0),
                                    stop=(kc == DC - 1),
                                )
                        st = ktstp.tile([P, SQ], F32R, tag="ktst")
                        nc.vector.tensor_copy(st[:, 0:w_cols], ps[:, 0:w_cols])
                        nc.gpsimd.dma_start(
                            out=kt_dram[j][:, half * SQ:half * SQ + w_cols],
                            in_=st[:, 0:w_cols],
                        )

            # ---- phase Q: QT resident, all pairs -------------------------
            with tc.tile_pool(name="qtres", bufs=1) as qtpool:
                qt = [
                    qtpool.tile([P, SQ], F32R, tag=f"qt{j}", name=f"qt{j}")
                    for j in range(NJ)
                ]
                with (
                    tc.tile_pool(name="xq", bufs=1) as xqp,
                    tc.tile_pool(name="wqp", bufs=1) as wqp,
                    tc.tile_pool(name="pq", bufs=2, space="PSUM") as pqp,
                ):
                    xq_cb = []
                    xq_ch = xqT.rearrange("(c p) s -> p c s", p=P)
                    for cb in range(2):
                        t = xqp.tile([P, DC, 512], F32R, tag=f"xq{cb}",
                                     name=f"xq{cb}")
                        eng = nc.sync if cb % 2 == 0 else nc.scalar
                        eng.dma_start(
                            out=t, in_=xq_ch[:, :, cb * 512:(cb + 1) * 512]
                        )
                        xq_cb.append(t)
                    wq_sb = wqp.tile([P, DC, HDH], F32R)
                    wq_ch = wq.rearrange("(c p) n -> p c n", p=P)
                    for kc in range(DC):
                        eng = nc.scalar if kc % 2 == 0 else nc.sync
                        eng.dma_start(out=wq_sb[:, kc, :], in_=wq_ch[:, kc, :])
                    for j in range(NJ):
                        ps = pqp.tile([P, SQ], F32, tag="pq")
                        for kc in range(DC):
                            lhsT = wq_sb[:, kc, j * P:(j + 1) * P]
                            for nh in range(2):
                                nc.tensor.matmul(
                                    ps[:, nh * 512:(nh + 1) * 512],
                                    lhsT,
                                    xq_cb[nh][:, kc, :],
                                    start=(kc == 0),
                                    stop=(kc == DC - 1),
                                )
                        nc.vector.tensor_scalar(
                            qt[j], ps, 0.125, bq8_sb[:, j:j + 1],
                            AOp.mult, AOp.add,
                        )

                # ---- attention -------------------------------------------
                with (
                    tc.tile_pool(name="otres", bufs=1) as otpool,
                    tc.tile_pool(name="wopre", bufs=1) as wopre,
                ):
                    wo_pre = wopre.tile([P, 16, P], F32R)
                    for jj in range(NJ):
                        nc.sync.dma_start(
                            out=wo_pre[:, jj, :],
                            in_=wo[jj * P:(jj + 1) * P, 0:P],
                        )
                        nc.sync.dma_start(
                            out=wo_pre[:, NJ + jj, :],
                            in_=wo[jj * P:(jj + 1) * P, P:2 * P],
                        )
                    ot = [
                        otpool.tile([P, SQ], F32R, tag=f"ot{j}", name=f"ot{j}")
                        for j in range(NJ)
                    ]
                    with (
                        tc.tile_pool(name="ktsb", bufs=2) as ktp,
                        tc.tile_pool(name="expp", bufs=3) as expp,
                        tc.tile_pool(name="lbp", bufs=2) as lbp,
                        tc.tile_pool(name="ps_s", bufs=1, space="PSUM") as pss,
                        tc.tile_pool(name="ps_o", bufs=1, space="PSUM") as pso,
                    ):
                        kt_tiles = {}

                        def load_kt(jj):
                            t = ktp.tile([P, kpad], F32R, tag="kt", name="kt_sb")
                            nc.sync.dma_start(out=t, in_=kt_dram[jj][:])
                            kt_tiles[jj] = t

                        load_kt(0)
                        for j in range(NJ):
                            if j + 1 < NJ:
                                load_kt(j + 1)  # prefetch next pair's KT
                            kt_sb = kt_tiles.pop(j)
                            ps_oa = pso.tile([VW, SQ], F32, tag="oa")
                            ps_ob = pso.tile([VW, SQ], F32, tag="ob")
                            ets = {}

                            def scores_exp(kc, j=j, kt_sb=kt_sb, ets=ets):
                                ps_s = pss.tile(
                                    [P, 2 * SQ], F32, tag="s", name="ps_s"
                                )
                                for hh in (0, 1):
                                    lhsT = kt_sb[hh * 64:(hh + 1) * 64,
                                                 kc * P:(kc + 1) * P]
                                    for nh in range(2):
                                        nc.tensor.matmul(
                                            ps_s[:, hh * SQ + nh * 512:
                                                 hh * SQ + (nh + 1) * 512],
                                            lhsT,
                                            qt[j][hh * 64:(hh + 1) * 64,
                                                  nh * 512:(nh + 1) * 512],
                                            tile_position=(hh * 64, 0),
                                        )
                                et = expp.tile(
                                    [P, 2 * SQ], F32R, tag="e", name="et"
                                )
                                nc.scalar.activation(
                                    et, ps_s, Exp,
                                    bias=maskb_sb[:, kc:kc + 1], scale=1.0,
                                )
                                ets[kc] = et

                            def pv(kc, j=j, ets=ets, ps_oa=ps_oa, ps_ob=ps_ob):
                                et = ets.pop(kc)
                                for hh, ps_o in ((0, ps_oa), (1, ps_ob)):
                                    vh = v_sb[:, kc, 2 * j + hh, :]  # [128,65]
                                    for nh in range(2):
                                        nc.tensor.matmul(
                                            ps_o[:, nh * 512:(nh + 1) * 512],
                                            vh,
                                            et[:, hh * SQ + nh * 512:
                                               hh * SQ + (nh + 1) * 512],
                                            start=(kc == 0),
                                            stop=(kc == kc_lim - 1),
                                        )

                            scores_exp(0)
                            scores_exp(1)
                            for kc in range(2, kc_lim):
                                scores_exp(kc)
                                pv(kc - 2)
                            pv(kc_lim - 2)
                            pv(kc_lim - 1)

                            # release PSUM fast: copy both heads to SBUF
                            # (incl. l rows); broadcast l, recip, then scale
                            cpA = lbp.tile([VW, SQ], F32, tag="cpA", bufs=2)
                            nc.vector.tensor_copy(cpA, ps_oa)
                            cpB = lbp.tile([VW, SQ], F32R, tag="cpB", bufs=2)
                            nc.vector.tensor_copy(cpB, ps_ob)
                            L_sb = lbp.tile([P, SQ], F32, tag="L", bufs=2)
                            for hh, rsrc in ((0, cpA), (1, cpB)):
                                rd = rdp.tile(
                                    [1, SQ], F32, tag="rd", name="rd"
                                )
                                nc.sync.dma_start(
                                    out=rd, in_=rsrc[DH:VW, :].bitcast(F32)
                                )
                                rd_b = bass.AP(
                                    tensor=rd.tensor, offset=rd.offset,
                                    ap=[[0, 64], rd.ap[-1]],
                                )
                                nc.sync.dma_start(
                                    out=L_sb[hh * 64:(hh + 1) * 64, :],
                                    in_=rd_b,
                                )
                            nc.vector.reciprocal_approx_fast(L_sb, L_sb)
                            nc.vector.tensor_mul(
                                ot[j][0:64, :], cpA[0:DH, :], L_sb[0:64, :]
                            )
                            nc.gpsimd.dma_start(
                                out=ot[j][64:128, :], in_=cpB[0:DH, :]
                            )
                            nc.vector.tensor_mul(
                                ot[j][64:128, :], ot[j][64:128, :],
                                L_sb[64:128, :],
                            )

                    # ---- output projection -------------------------------
                    with (
                        tc.tile_pool(name="wop", bufs=8) as wop,
                        tc.tile_pool(name="ytp", bufs=3) as ytp,
                        tc.tile_pool(name="py", bufs=2, space="PSUM") as pyp,
                    ):
                        yt_ch = yT.rearrange("(c p) s -> c p s", p=P)
                        for dc in range(DC):
                            ps = pyp.tile([P, SQ], F32, tag="py")
                            for j in range(NJ):
                                if dc < 2:
                                    wo_t = wo_pre[:, dc * NJ + j, :]
                                else:
                                    wo_t = wop.tile([P, P], F32R, tag="wo")
                                    nc.scalar.dma_start(
                                        out=wo_t,
                                        in_=wo[j * P:(j + 1) * P,
                                               dc * P:(dc + 1) * P],
                                    )
                                for nh in range(2):
                                    nc.tensor.matmul(
                                        ps[:, nh * 512:(nh + 1) * 512],
                                        wo_t,
                                        ot[j][:, nh * 512:(nh + 1) * 512],
                                        start=(j == 0),
                                        stop=(j == NJ - 1),
                                    )
                            yt_sb = ytp.tile([P, SQ], F32, tag="yt")
                            nc.vector.tensor_scalar(
                                yt_sb, ps, bo2_sb[:, dc:dc + 1], None, AOp.add
                            )
                            nc.gpsimd.dma_start(out=yt_ch[dc], in_=yt_sb)

    nc.compile()
    return nc


def _prepare(x_Q, x_K, x_V, src_batch_lens, Wq, bq, Wk, bk, Wv, bv, Wo, bo):
    x_Q = np.asarray(x_Q, dtype=np.float32)
    x_K = np.asarray(x_K, dtype=np.float32)
    x_V = np.asarray(x_V, dtype=np.float32)
    lens = np.asarray(src_batch_lens)
    Wq = np.ascontiguousarray(np.asarray(Wq, dtype=np.float32))
    Wk = np.ascontiguousarray(np.asarray(Wk, dtype=np.float32))
    Wv = np.ascontiguousarray(np.asarray(Wv, dtype=np.float32))
    Wo = np.ascontiguousarray(np.asarray(Wo, dtype=np.float32))
    bq = np.asarray(bq, dtype=np.float32)
    bv = np.asarray(bv, dtype=np.float32)
    bo = np.asarray(bo, dtype=np.float32)

    maxlen = int(np.max(lens))
    maxlen = max(1, min(S, maxlen))
    kc_lim = (maxlen + P - 1) // P

    bo2_full = (bv @ Wo + bo).astype(np.float32)
    bo2 = np.ascontiguousarray(bo2_full.reshape(DC, P).T)
    bq8 = np.ascontiguousarray((bq / 8.0).reshape(NJ, P).T)

    in_maps = []
    for c in range(8):
        b, hh = c // 2, c % 2
        q0 = hh * SQ
        k_idx = np.arange(S)
        mvec = np.where(k_idx < int(lens[b]), 0.0, MASK_NEG).astype(np.float32)
        in_maps.append({
            "xqT": np.ascontiguousarray(x_Q[b, q0:q0 + SQ, :].T),
            "xkT": np.ascontiguousarray(x_K[b].T),
            "xvT": np.ascontiguousarray(x_V[b].T),
            "wq": Wq, "wk": Wk, "wv": Wv, "wo": Wo,
            "bq8": bq8, "bo2": bo2,
            "maskb": np.ascontiguousarray(mvec.reshape(KC, P).T),
        })
    return kc_lim, in_maps


def _build_in_maps(inputs):
    return _prepare(**inputs)[1]


def kernel(x_Q, x_K, x_V, src_batch_lens, Wq, bq, Wk, bk, Wv, bv, Wo, bo):
    kc_lim, in_maps = _prepare(x_Q, x_K, x_V, src_batch_lens,
                               Wq, bq, Wk, bk, Wv, bv, Wo, bo)
    if kc_lim not in _CACHE:
        _CACHE[kc_lim] = build_bass(kc_lim)
    nc = _CACHE[kc_lim]

    res = run_bass_kernel_spmd(nc, in_maps, core_ids=list(range(8)))

    out = np.empty((B, S, D), dtype=np.float32)
    for c in range(8):
        b, hh = c // 2, c % 2
        q0 = hh * SQ
        out[b, q0:q0 + SQ, :] = res.results[c]["yT"].T
    return out



# revision 4
# speedup vs baseline: 1.2366x; 1.2366x over previous
"""Trainium2 Bass kernel for nn_MultiHeadAttention (B=4, S=2048, D=1024, H=16, DH=64).

Sharding: 8 cores = 4 batches x 2 query-halves. Each core computes, for its
(batch b, query half): Q/K/V projections, masked softmax attention over the
full key length, and the output projection, entirely on-device.

v2 vs baseline: all matmul operands bf16 (host-cast), KT kept SBUF-resident
(no DRAM spill/reload), K projection trimmed to the padded key length,
head-granular score PSUM tiles double-buffered so the PE never waits for the
Exp drain, and DMA staging sized so each phase's inputs arrive during the
previous phase's compute (PE stays HAM-warm).

Device-side layout is fully transposed (feature-major) so every matmul has its
contraction on the partition dim:
  QT = Wq^T xqT / 8         [HDH, Sq]   (1/8 score scale + bq folded in)
  KT = Wk^T xkT             [HDH, klen] (SBUF-resident bf16)
  V  = (Wv^T xvT)^T         [klen, HDH] stored interleaved per head as
                                        [s, h, 65] with a ones column, so the
                                        PV matmul's PSUM row 64 accumulates
                                        l = sum_k exp(scoresT) for free
  scoresT_h = KT_h^T QT_h   [klen, Sq]  (per head; 2 heads packed in PE rows)
  expT = exp(scoresT + mask_bias[k])    (no row-max: |scores| <= ~4)
  outT_h = V_h^T expT / l   [DH, Sq]
  yT = Wo^T outT + bo'      [D, Sq]
Key-padding mask enters as a per-partition bias (0 / -40) on the Exp
activation. bk is dropped (softmax-invariant); bv,bo fold into bo' = bv@Wo+bo
host-side (exact, since softmax rows sum to 1).
"""

import sys
import numpy as np
import ml_dtypes

if "/opt/trn_rl_repo" not in sys.path:
    sys.path.insert(0, "/opt/trn_rl_repo")

import concourse.bass as bass
import concourse.mybir as mybir
import concourse.tile as tile
from concourse import bacc
from concourse.bass_utils import run_bass_kernel_spmd

B, S, D = 4, 2048, 1024
H, DH = 16, 64
HDH = H * DH                      # 1024
SQ = S // 2                       # 1024 queries per core
P = 128
DC = D // P                       # 8 contraction chunks
NJ = 8                            # head pairs (2 heads x 64 rows = 128)
KC = S // P                       # max key chunks
VW = DH + 1                       # 65: V columns per head + ones column
F32 = mybir.dt.float32
BF16 = mybir.dt.bfloat16
MASK_NEG = -40.0
BF = ml_dtypes.bfloat16

_CACHE = {}


def build_bass(kc_lim=KC):
    nc = bacc.Bacc("TRN2", target_bir_lowering=False, debug=False)
    klen = kc_lim * P                # keys actually attended (rest fully masked)
    # key blocks for K projection / staging: 512-wide, last may be short
    kblk = []
    c = 0
    while c < klen:
        w = min(512, klen - c)
        kblk.append((c, w))
        c += w

    xqT = nc.dram_tensor("xqT", [D, SQ], BF16, kind="ExternalInput").ap()
    xkT = nc.dram_tensor("xkT", [D, klen], BF16, kind="ExternalInput").ap()
    xvT = nc.dram_tensor("xvT", [D, klen], BF16, kind="ExternalInput").ap()
    wq = nc.dram_tensor("wq", [D, HDH], BF16, kind="ExternalInput").ap()
    wk = nc.dram_tensor("wk", [D, HDH], BF16, kind="ExternalInput").ap()
    wv = nc.dram_tensor("wv", [D, HDH], BF16, kind="ExternalInput").ap()
    wo = nc.dram_tensor("wo", [HDH, D], BF16, kind="ExternalInput").ap()
    bq8 = nc.dram_tensor("bq8", [P, NJ], F32, kind="ExternalInput").ap()
    bo2 = nc.dram_tensor("bo2", [P, DC], F32, kind="ExternalInput").ap()
    maskb = nc.dram_tensor("maskb", [P, kc_lim], F32, kind="ExternalInput").ap()
    yT = nc.dram_tensor("yT", [D, SQ], BF16, kind="ExternalOutput").ap()

    Exp = mybir.ActivationFunctionType.Exp
    AOp = mybir.AluOpType

    with tile.TileContext(nc) as tc:
        with (
            tc.tile_pool(name="const", bufs=1) as cpool,
            tc.tile_pool(name="ktres", bufs=1) as ktpool,
            tc.tile_pool(name="vres", bufs=1) as vpool,
            tc.tile_pool(name="qtres", bufs=1) as qtpool,
            tc.tile_pool(name="otres", bufs=1) as otpool,
            tc.tile_pool(name="rdram", bufs=2, space="DRAM") as rdp,
        ):
            maskb_sb = cpool.tile([P, kc_lim], F32)
            nc.sync.dma_start(out=maskb_sb, in_=maskb)
            bq8_sb = cpool.tile([P, NJ], F32)
            nc.sync.dma_start(out=bq8_sb, in_=bq8)
            bo2_sb = cpool.tile([P, DC], F32)
            nc.sync.dma_start(out=bo2_sb, in_=bo2)

            kt_sb = ktpool.tile([P, NJ, klen], BF16)          # 28KB/part
            v_sb = vpool.tile([P, kc_lim, H, VW], BF16)       # ~29KB/part
            qt = qtpool.tile([P, NJ, SQ], BF16)               # 16KB/part
            ot = otpool.tile([P, NJ, SQ], BF16)               # 16KB/part

            # ones column of V, set once: [:, :, :, 64]
            nc.vector.memset(v_sb[:, :, :, DH:VW], 1.0)

            # ---- phase Q: QT resident, all pairs -------------------------
            with (
                tc.tile_pool(name="xq", bufs=1) as xqp,
                tc.tile_pool(name="wqp", bufs=1) as wqp,
                tc.tile_pool(name="pq", bufs=2, space="PSUM") as pqp,
            ):
                xq_sb = xqp.tile([P, DC, SQ], BF16)
                xq_ch = xqT.rearrange("(c p) s -> p c s", p=P)
                wq_sb = wqp.tile([P, DC, HDH], BF16)
                wq_ch = wq.rearrange("(c p) n -> p c n", p=P)
                for kc in range(DC):
                    eng = nc.sync if kc % 2 == 0 else nc.scalar
                    eng.dma_start(out=wq_sb[:, kc, :], in_=wq_ch[:, kc, :])
                for kc in range(DC):
                    eng = nc.scalar if kc % 2 == 0 else nc.sync
                    eng.dma_start(out=xq_sb[:, kc, :], in_=xq_ch[:, kc, :])
                for j in range(NJ):
                    ps = pqp.tile([P, SQ], F32, tag="pq")
                    for kc in range(DC):
                        lhsT = wq_sb[:, kc, j * P:(j + 1) * P]
                        for nh in range(2):
                            nc.tensor.matmul(
                                ps[:, nh * 512:(nh + 1) * 512],
                                lhsT,
                                xq_sb[:, kc, nh * 512:(nh + 1) * 512],
                                start=(kc == 0),
                                stop=(kc == DC - 1),
                            )
                    nc.vector.tensor_scalar(
                        qt[:, j, :], ps, 0.125, bq8_sb[:, j:j + 1],
                        AOp.mult, AOp.add,
                    )

            # ---- phase K: KT -> SBUF bf16, all pairs ---------------------
            with (
                tc.tile_pool(name="xk", bufs=2) as xkp,
                tc.tile_pool(name="wkp", bufs=1) as wkp,
                tc.tile_pool(name="pk", bufs=3, space="PSUM") as pkp,
            ):
                wk_sb = wkp.tile([P, DC, HDH], BF16)
                wk_ch = wk.rearrange("(c p) n -> p c n", p=P)
                for kc in range(DC):
                    eng = nc.sync if kc % 2 == 0 else nc.scalar
                    eng.dma_start(out=wk_sb[:, kc, :], in_=wk_ch[:, kc, :])
                xk_ch = xkT.rearrange("(c p) s -> p c s", p=P)
                for kb, (c0, w) in enumerate(kblk):
                    xt = xkp.tile([P, DC, 512], BF16, tag="xk", name=f"xk{kb}")
                    eng = nc.scalar if kb % 2 == 0 else nc.sync
                    eng.dma_start(out=xt[:, :, 0:w], in_=xk_ch[:, :, c0:c0 + w])
                    for j in range(NJ):
                        ps = pkp.tile([P, 512], F32, tag="pk")
                        for kc in range(DC):
                            nc.tensor.matmul(
                                ps[:, 0:w],
                                wk_sb[:, kc, j * P:(j + 1) * P],
                                xt[:, kc, 0:w],
                                start=(kc == 0),
                                stop=(kc == DC - 1),
                            )
                        nc.vector.tensor_copy(
                            kt_sb[:, j, c0:c0 + w], ps[:, 0:w]
                        )

            # ---- phase V: V = (Wv^T xvT)^T interleaved, bf16 -------------
            with (
                tc.tile_pool(name="xv", bufs=2) as xvp,
                tc.tile_pool(name="wvp", bufs=1) as wvp,
                tc.tile_pool(name="pv", bufs=2, space="PSUM") as pvp,
            ):
                wv_sb = wvp.tile([P, DC, HDH], BF16)
                wv_ch = wv.rearrange("(c p) n -> p c n", p=P)
                for kc in range(DC):
                    eng = nc.sync if kc % 2 == 0 else nc.scalar
                    eng.dma_start(out=wv_sb[:, kc, :], in_=wv_ch[:, kc, :])
                xv_ch = xvT.rearrange("(c p) s -> p c s", p=P)
                xv_cb = {}
                for cb in range((kc_lim + 3) // 4):
                    t = xvp.tile([P, DC, 512], BF16, tag="xv", name=f"xv{cb}")
                    w = min(512, klen - cb * 512)
                    eng = nc.scalar if cb % 2 == 0 else nc.sync
                    eng.dma_start(
                        out=t[:, :, 0:w], in_=xv_ch[:, :, cb * 512:cb * 512 + w]
                    )
                    xv_cb[cb] = t
                for sc in range(kc_lim):
                    cb, scl = sc // 4, sc % 4
                    ps = pvp.tile([P, HDH], F32, tag="pv")
                    for kc in range(DC):
                        lhsT = xv_cb[cb][:, kc, scl * P:(scl + 1) * P]
                        for nh in range(2):
                            nc.tensor.matmul(
                                ps[:, nh * 512:(nh + 1) * 512],
                                lhsT,
                                wv_sb[:, kc, nh * 512:(nh + 1) * 512],
                                start=(kc == 0),
                                stop=(kc == DC - 1),
                            )
                    # strided copy into the interleaved bf16 layout
                    nc.vector.tensor_copy(
                        v_sb[:, sc, :, 0:DH],
                        ps.rearrange("p (h d) -> p h d", d=DH),
                    )

            # ---- attention, per head pair --------------------------------
            with tc.tile_pool(name="wopre", bufs=1) as wopre:
                # preload Wo during attention (16KB/part)
                wo_sb = wopre.tile([P, NJ, D], BF16)
                wo_ch = wo.rearrange("(j p) d -> p j d", p=P)
                for j in range(NJ):
                    nc.sync.dma_start(out=wo_sb[:, j, :], in_=wo_ch[:, j, :])

                with (
                    tc.tile_pool(name="expp", bufs=4) as expp,
                    tc.tile_pool(name="lbp", bufs=2) as lbp,
                    tc.tile_pool(name="ps_s", bufs=2, space="PSUM") as pss,
                    tc.tile_pool(name="ps_o", bufs=1, space="PSUM") as pso,
                ):
                  for j in range(NJ):
                    ps_oa = pso.tile([VW, SQ], F32, tag="oa")
                    ps_ob = pso.tile([VW, SQ], F32, tag="ob")
                    ets = {}

                    def scores_exp(kc, hh, j=j, ets=ets):
                        ps_s = pss.tile([P, SQ], F32, tag="s", name="ps_s")
                        # head (2j+hh) lives in kt_sb[:, j] partitions
                        # hh*64..hh*64+63
                        lhsT = kt_sb[hh * 64:(hh + 1) * 64, j,
                                     kc * P:(kc + 1) * P]
                        for nh in range(2):
                            nc.tensor.matmul(
                                ps_s[:, nh * 512:(nh + 1) * 512],
                                lhsT,
                                qt[hh * 64:(hh + 1) * 64, j,
                                   nh * 512:(nh + 1) * 512],
                                tile_position=(hh * 64, 0),
                            )
                        et = expp.tile([P, SQ], BF16, tag="et", name="et")
                        nc.scalar.activation(
                            et, ps_s, Exp,
                            bias=maskb_sb[:, kc:kc + 1], scale=1.0,
                        )
                        ets[(kc, hh)] = et

                    def pv(kc, hh, j=j, ets=ets, ps_oa=ps_oa, ps_ob=ps_ob):
                        et = ets.pop((kc, hh))
                        ps_o = ps_oa if hh == 0 else ps_ob
                        vh = v_sb[:, kc, 2 * j + hh, :]  # [128, 65]
                        for nh in range(2):
                            nc.tensor.matmul(
                                ps_o[:, nh * 512:(nh + 1) * 512],
                                vh,
                                et[:, nh * 512:(nh + 1) * 512],
                                start=(kc == 0),
                                stop=(kc == kc_lim - 1),
                            )

                    scores_exp(0, 0)
                    scores_exp(0, 1)
                    for kc in range(1, kc_lim):
                        scores_exp(kc, 0)
                        pv(kc - 1, 0)
                        scores_exp(kc, 1)
                        pv(kc - 1, 1)
                    pv(kc_lim - 1, 0)
                    pv(kc_lim - 1, 1)

                    # release PSUM fast: copy both heads to SBUF (incl. l
                    # rows); broadcast l via DRAM roundtrip, recip, scale
                    cpA = lbp.tile([VW, SQ], F32, tag="cpA", bufs=2)
                    nc.vector.tensor_copy(cpA, ps_oa)
                    cpB = lbp.tile([VW, SQ], F32, tag="cpB", bufs=2)
                    nc.vector.tensor_copy(cpB, ps_ob)
                    L_sb = lbp.tile([P, SQ], F32, tag="L", bufs=2)
                    for hh, rsrc in ((0, cpA), (1, cpB)):
                        rd = rdp.tile([1, SQ], F32, tag="rd", name="rd")
                        nc.sync.dma_start(out=rd, in_=rsrc[DH:VW, :])
                        rd_b = bass.AP(
                            tensor=rd.tensor, offset=rd.offset,
                            ap=[[0, 64], rd.ap[-1]],
                        )
                        nc.sync.dma_start(
                            out=L_sb[hh * 64:(hh + 1) * 64, :], in_=rd_b
                        )
                    nc.vector.reciprocal_approx_fast(L_sb, L_sb)
                    nc.vector.tensor_mul(
                        ot[0:64, j, :], cpA[0:DH, :], L_sb[0:64, :]
                    )
                    otf = lbp.tile([P, SQ], F32, tag="otf", bufs=2)
                    nc.gpsimd.dma_start(out=otf[64:128, :], in_=cpB[0:DH, :])
                    nc.vector.tensor_mul(
                        ot[64:128, j, :], otf[64:128, :], L_sb[64:128, :]
                    )

                # ---- output projection -----------------------------------
                with (
                    tc.tile_pool(name="ytp", bufs=3) as ytp,
                    tc.tile_pool(name="py", bufs=2, space="PSUM") as pyp,
                ):
                    yt_ch = yT.rearrange("(c p) s -> c p s", p=P)
                    for dc in range(DC):
                        ps = pyp.tile([P, SQ], F32, tag="py")
                        for j in range(NJ):
                            lhsT = wo_sb[:, j, dc * P:(dc + 1) * P]
                            for nh in range(2):
                                nc.tensor.matmul(
                                    ps[:, nh * 512:(nh + 1) * 512],
                                    lhsT,
                                    ot[:, j, nh * 512:(nh + 1) * 512],
                                    start=(j == 0),
                                    stop=(j == NJ - 1),
                                )
                        yt_sb = ytp.tile([P, SQ], BF16, tag="yt")
                        nc.vector.tensor_scalar(
                            yt_sb, ps, bo2_sb[:, dc:dc + 1], None, AOp.add
                        )
                        nc.gpsimd.dma_start(out=yt_ch[dc], in_=yt_sb)

    nc.compile()
    return nc


def _prepare(x_Q, x_K, x_V, src_batch_lens, Wq, bq, Wk, bk, Wv, bv, Wo, bo):
    x_Q = np.asarray(x_Q, dtype=np.float32)
    x_K = np.asarray(x_K, dtype=np.float32)
    x_V = np.asarray(x_V, dtype=np.float32)
    lens = np.asarray(src_batch_lens)
    Wq = np.asarray(Wq, dtype=np.float32)
    Wk = np.asarray(Wk, dtype=np.float32)
    Wv = np.asarray(Wv, dtype=np.float32)
    Wo = np.asarray(Wo, dtype=np.float32)
    bq = np.asarray(bq, dtype=np.float32)
    bv = np.asarray(bv, dtype=np.float32)
    bo = np.asarray(bo, dtype=np.float32)

    maxlen = int(np.max(lens))
    maxlen = max(1, min(S, maxlen))
    kc_lim = (maxlen + P - 1) // P
    klen = kc_lim * P

    bo2_full = (bv @ Wo + bo).astype(np.float32)
    bo2 = np.ascontiguousarray(bo2_full.reshape(DC, P).T)
    bq8 = np.ascontiguousarray((bq / 8.0).reshape(NJ, P).T)
    wq_b = np.ascontiguousarray(Wq.astype(BF))
    wk_b = np.ascontiguousarray(Wk.astype(BF))
    wv_b = np.ascontiguousarray(Wv.astype(BF))
    wo_b = np.ascontiguousarray(Wo.astype(BF))

    in_maps = []
    for c in range(8):
        b, hh = c // 2, c % 2
        q0 = hh * SQ
        k_idx = np.arange(klen)
        mvec = np.where(k_idx < int(lens[b]), 0.0, MASK_NEG).astype(np.float32)
        in_maps.append({
            "xqT": np.ascontiguousarray(x_Q[b, q0:q0 + SQ, :].T.astype(BF)),
            "xkT": np.ascontiguousarray(x_K[b, 0:klen, :].T.astype(BF)),
            "xvT": np.ascontiguousarray(x_V[b, 0:klen, :].T.astype(BF)),
            "wq": wq_b, "wk": wk_b, "wv": wv_b, "wo": wo_b,
            "bq8": bq8, "bo2": bo2,
            "maskb": np.ascontiguousarray(mvec.reshape(kc_lim, P).T),
        })
    return kc_lim, in_maps


def _build_in_maps(inputs):
    return _prepare(**inputs)[1]


def kernel(x_Q, x_K, x_V, src_batch_lens, Wq, bq, Wk, bk, Wv, bv, Wo, bo):
    kc_lim, in_maps = _prepare(x_Q, x_K, x_V, src_batch_lens,
                               Wq, bq, Wk, bk, Wv, bv, Wo, bo)
    if kc_lim not in _CACHE:
        _CACHE[kc_lim] = build_bass(kc_lim)
    nc = _CACHE[kc_lim]

    res = run_bass_kernel_spmd(nc, in_maps, core_ids=list(range(8)))

    out = np.empty((B, S, D), dtype=np.float32)
    for c in range(8):
        b, hh = c // 2, c % 2
        q0 = hh * SQ
        out[b, q0:q0 + SQ, :] = res.results[c]["yT"].T.astype(np.float32)
    return out


# revision 5
# speedup vs baseline: 1.5541x; 1.2568x over previous
"""Trainium2 Bass kernel for nn_MultiHeadAttention (B=4, S=2048, D=1024, H=16, DH=64).

Sharding (v3, head-split): 8 cores x 2 heads each. Every core processes ALL
batches/queries for its 2 heads: Q/K/V projections with column-sliced
weights, masked softmax attention with per-batch key-length trimming baked
into the program, and a row-parallel output projection producing a partial
y that the host sums across cores during unshard (plus the bias).

Benefits vs batch-split: no duplicated K/V projections, per-batch key
lengths shorten both the PE score/PV streams and the ACT exp stream
(sum(ceil(len_b/128)) chunks instead of 4*max), and all cores do identical
work (no straggler batch).

Device-side layout is feature-major so every matmul contracts on the
partition dim. Per core (2 heads stacked as partition halves 0-63 / 64-127):
  QT = Wq2^T xqT / 8        [128, B*S]    (1/8 scale + bq folded in)
  KT_b = Wk2^T xkT_b        [128, klen_b] (SBUF-resident bf16)
  V_b  = (Wv2^T xvT_b)^T    [klen_b, 2, 65] interleaved, ones column at 64
                                          so PV's PSUM row 64 accumulates l
  scoresT_h = KT_h^T QT_h   [klen_b, 1024-query window]  per (h, nh)
  expT = exp(scoresT + mask_bias[k])      (no row-max: |scores| <= ~5)
  outT_h = V_h^T expT / l   [64, window]
  yT_partial = Wo2^T outT   [D, B*S]      (host adds partials + bias)
All matmul operands bf16 (PSUM fp32). Attention is emitted interleaved with
the next batch's projection matmuls (fill queue) so the PE never idles while
the ACT engine streams the exps (keeps the HAM clock-gate warm).
"""

import sys
from collections import deque
import numpy as np
import ml_dtypes

if "/opt/trn_rl_repo" not in sys.path:
    sys.path.insert(0, "/opt/trn_rl_repo")

import concourse.bass as bass
import concourse.mybir as mybir
import concourse.tile as tile
from concourse import bacc
from concourse.bass_utils import run_bass_kernel_spmd

B, S, D = 4, 2048, 1024
H, DH = 16, 64
BS = B * S                        # 8192 query tokens
P = 128
DC = D // P                       # 8 contraction chunks over D
VW = DH + 1                       # 65: V columns per head + ones column
F32 = mybir.dt.float32
BF16 = mybir.dt.bfloat16
MASK_NEG = -40.0
BF = ml_dtypes.bfloat16

_CACHE = {}


def build_bass(kcs):
    """kcs: per-batch key-chunk counts, e.g. (11, 13, 14, 10)."""
    nc = bacc.Bacc("TRN2", target_bir_lowering=False, debug=False)
    kcs = list(kcs)
    klens = [k * P for k in kcs]
    ktok = sum(klens)                 # key/value tokens kept overall
    cbase = [sum(kcs[:b]) for b in range(B)]      # mask column base
    tbase = [sum(klens[:b]) for b in range(B)]    # key-token base
    kmax = max(klens)
    kcmax = max(kcs)

    xqT = nc.dram_tensor("xqT", [D, BS], BF16, kind="ExternalInput").ap()
    xkT = nc.dram_tensor("xkT", [D, ktok], BF16, kind="ExternalInput").ap()
    xvT = nc.dram_tensor("xvT", [D, ktok], BF16, kind="ExternalInput").ap()
    wq2 = nc.dram_tensor("wq2", [D, P], BF16, kind="ExternalInput").ap()
    wk2 = nc.dram_tensor("wk2", [D, P], BF16, kind="ExternalInput").ap()
    wv2 = nc.dram_tensor("wv2", [D, P], BF16, kind="ExternalInput").ap()
    wo2 = nc.dram_tensor("wo2", [P, D], BF16, kind="ExternalInput").ap()
    bq2 = nc.dram_tensor("bq2", [P, 1], F32, kind="ExternalInput").ap()
    maskb = nc.dram_tensor("maskb", [P, sum(kcs)], F32,
                           kind="ExternalInput").ap()
    yT = nc.dram_tensor("yT", [D, BS], BF16, kind="ExternalOutput").ap()

    Exp = mybir.ActivationFunctionType.Exp
    AOp = mybir.AluOpType

    xq_ch = xqT.rearrange("(c p) s -> p c s", p=P)
    xk_ch = xkT.rearrange("(c p) s -> p c s", p=P)
    xv_ch = xvT.rearrange("(c p) s -> p c s", p=P)
    yt_ch = yT.rearrange("(c p) s -> c p s", p=P)

    with tile.TileContext(nc) as tc:
        with (
            tc.tile_pool(name="const", bufs=1) as cpool,
            tc.tile_pool(name="wts", bufs=1) as wpool,
            tc.tile_pool(name="ktp", bufs=2) as ktpool,
            tc.tile_pool(name="vp", bufs=2) as vpool,
            tc.tile_pool(name="qtp", bufs=2) as qtpool,
            tc.tile_pool(name="otp", bufs=2) as otpool,
            tc.tile_pool(name="xqs", bufs=4) as xqpool,
            tc.tile_pool(name="xks", bufs=4) as xkpool,
            tc.tile_pool(name="xvs", bufs=4) as xvpool,
            tc.tile_pool(name="expp", bufs=4) as expp,
            tc.tile_pool(name="lbp", bufs=2) as lbp,
            tc.tile_pool(name="ytp", bufs=3) as ytp,
            tc.tile_pool(name="rdram", bufs=4, space="DRAM") as rdp,
            tc.tile_pool(name="ps_s", bufs=2, space="PSUM") as pss,
            tc.tile_pool(name="ps_o", bufs=1, space="PSUM") as pso,
            tc.tile_pool(name="ps_p", bufs=2, space="PSUM") as psp,
        ):
            maskb_sb = cpool.tile([P, sum(kcs)], F32)
            nc.sync.dma_start(out=maskb_sb, in_=maskb)
            bq2_sb = cpool.tile([P, 1], F32)
            nc.sync.dma_start(out=bq2_sb, in_=bq2)
            wq_sb = wpool.tile([P, DC, P], BF16)
            nc.scalar.dma_start(
                out=wq_sb, in_=wq2.rearrange("(c p) n -> p c n", p=P)
            )
            wk_sb = wpool.tile([P, DC, P], BF16)
            nc.scalar.dma_start(
                out=wk_sb, in_=wk2.rearrange("(c p) n -> p c n", p=P)
            )
            wv_sb = wpool.tile([P, DC, P], BF16)
            nc.scalar.dma_start(
                out=wv_sb, in_=wv2.rearrange("(c p) n -> p c n", p=P)
            )
            wo_sb = wpool.tile([P, D], BF16)
            nc.scalar.dma_start(out=wo_sb, in_=wo2)

            # ---------- staging loads (issued early; slots throttle) ------
            xq_t, xk_t, xv_t = {}, {}, {}

            def load_batch(b):
                # xk / xv in 512-key chunks; xq in 512-query chunks
                nchunk = (klens[b] + 511) // 512
                for c in range(nchunk):
                    c0 = tbase[b] + c * 512
                    w = min(512, klens[b] - c * 512)
                    t = xkpool.tile([P, DC, 512], BF16, tag="xk",
                                    name=f"xk{b}_{c}")
                    nc.sync.dma_start(
                        out=t[:, :, 0:w], in_=xk_ch[:, :, c0:c0 + w]
                    )
                    xk_t[(b, c)] = t
                    t = xvpool.tile([P, DC, 512], BF16, tag="xv",
                                    name=f"xv{b}_{c}")
                    nc.sync.dma_start(
                        out=t[:, :, 0:w], in_=xv_ch[:, :, c0:c0 + w]
                    )
                    xv_t[(b, c)] = t
                for c in range(4):
                    q0 = b * S + c * 512
                    t = xqpool.tile([P, DC, 512], BF16, tag="xq",
                                    name=f"xq{b}_{c}")
                    nc.sync.dma_start(out=t, in_=xq_ch[:, :, q0:q0 + 512])
                    xq_t[(b, c)] = t

            # ---------- projection emitters (run as fill work) ------------
            kt_b, v_b, qt_b, ot_b = {}, {}, {}, {}

            def kproj_thunks(b):
                kt = ktpool.tile([P, kmax], BF16, tag="kt", name=f"kt{b}")
                kt_b[b] = kt
                nchunk = (klens[b] + 511) // 512

                def grp(c):
                    def f():
                        w = min(512, klens[b] - c * 512)
                        xt = xk_t.pop((b, c))
                        ps = psp.tile([P, 512], F32, tag="pp")
                        for kc in range(DC):
                            nc.tensor.matmul(
                                ps[:, 0:w],
                                wk_sb[:, kc, :],
                                xt[:, kc, 0:w],
                                start=(kc == 0),
                                stop=(kc == DC - 1),
                            )
                        nc.vector.tensor_copy(
                            kt[:, c * 512:c * 512 + w], ps[:, 0:w]
                        )
                    return f
                return [grp(c) for c in range(nchunk)]

            def vproj_thunks(b):
                v = vpool.tile([P, kcmax, 2, VW], BF16, tag="v", name=f"v{b}")
                v_b[b] = v

                def ones():
                    nc.vector.memset(v[:, :, :, DH:VW], 1.0)

                def grp(sc):
                    def f():
                        xt = xv_t[(b, sc // 4)]
                        scl = sc % 4
                        if sc == kcs[b] - 1 or scl == 3:
                            xv_t.pop((b, sc // 4), None)
                        ps = psp.tile([P, 512], F32, tag="pp")
                        for kc in range(DC):
                            nc.tensor.matmul(
                                ps[:, 0:P],
                                xt[:, kc, scl * P:(scl + 1) * P],
                                wv_sb[:, kc, :],
                                start=(kc == 0),
                                stop=(kc == DC - 1),
                            )
                        nc.vector.tensor_copy(
                            v[:, sc, :, 0:DH],
                            ps[:, 0:P].rearrange("p (h d) -> p h d", d=DH),
                        )
                    return f
                return [ones] + [grp(sc) for sc in range(kcs[b])]

            def qproj_thunks(b):
                qt = qtpool.tile([P, S], BF16, tag="qt", name=f"qt{b}")
                qt_b[b] = qt

                def grp(c):
                    def f():
                        xt = xq_t.pop((b, c))
                        ps = psp.tile([P, 512], F32, tag="pp")
                        for kc in range(DC):
                            nc.tensor.matmul(
                                ps,
                                wq_sb[:, kc, :],
                                xt[:, kc, :],
                                start=(kc == 0),
                                stop=(kc == DC - 1),
                            )
                        nc.vector.tensor_scalar(
                            qt[:, c * 512:(c + 1) * 512], ps, 0.125, bq2_sb,
                            AOp.mult, AOp.add,
                        )
                    return f
                return [grp(c) for c in range(4)]

            def oproj_thunks(b):
                ot = ot_b[b]

                def grp(dc, c):
                    def f():
                        ps = psp.tile([P, 512], F32, tag="pp")
                        nc.tensor.matmul(
                            ps,
                            wo_sb[:, dc * P:(dc + 1) * P],
                            ot[:, c * 512:(c + 1) * 512],
                            start=True,
                            stop=True,
                        )
                        yt = ytp.tile([P, 512], BF16, tag="yt")
                        nc.vector.tensor_copy(yt, ps)
                        nc.gpsimd.dma_start(
                            out=yt_ch[dc, :, b * S + c * 512:
                                      b * S + (c + 1) * 512],
                            in_=yt,
                        )
                    return f
                return [grp(dc, c) for c in range(4) for dc in range(DC)]

            fills = deque()

            def drain(n):
                k = 0
                while fills and k < n:
                    fills.popleft()()
                    k += 1

            # ---------- attention ----------------------------------------
            def attention(b):
                kt, v, qt, kcb = kt_b[b], v_b[b], qt_b[b], kcs[b]
                ot = otpool.tile([P, S], BF16, tag="ot", name=f"ot{b}")
                ot_b[b] = ot
                for h in range(2):
                    for nh in range(2):
                        q0 = nh * 1024
                        ps_o = pso.tile([VW, 1024], F32, tag="po")
                        ets = {}

                        def scores_exp(kc):
                            ps_s = pss.tile([P, 1024], F32, tag="s",
                                            name="ps_s")
                            lhsT = kt[h * 64:(h + 1) * 64,
                                      kc * P:(kc + 1) * P]
                            for n2 in range(2):
                                nc.tensor.matmul(
                                    ps_s[:, n2 * 512:(n2 + 1) * 512],
                                    lhsT,
                                    qt[h * 64:(h + 1) * 64,
                                       q0 + n2 * 512:q0 + (n2 + 1) * 512],
                                    tile_position=(h * 64, 0),
                                )
                            et = expp.tile([P, 1024], BF16, tag="et",
                                           name="et")
                            ck = cbase[b] + kc
                            nc.scalar.activation(
                                et, ps_s, Exp,
                                bias=maskb_sb[:, ck:ck + 1], scale=1.0,
                            )
                            ets[kc] = et

                        def pv(kc):
                            et = ets.pop(kc)
                            vh = v[:, kc, h, :]      # [128, 65]
                            for n2 in range(2):
                                nc.tensor.matmul(
                                    ps_o[:, n2 * 512:(n2 + 1) * 512],
                                    vh,
                                    et[:, n2 * 512:(n2 + 1) * 512],
                                    start=(kc == 0),
                                    stop=(kc == kcb - 1),
                                )

                        scores_exp(0)
                        for kc in range(1, kcb):
                            scores_exp(kc)
                            pv(kc - 1)
                            drain(5)
                        pv(kcb - 1)

                        # normalize: 1/l broadcast via DRAM roundtrip
                        cp = lbp.tile([VW, 1024], F32, tag="cp", bufs=2)
                        nc.vector.tensor_copy(cp, ps_o)
                        nc.vector.reciprocal_approx_fast(
                            cp[DH:VW, :], cp[DH:VW, :]
                        )
                        rd = rdp.tile([1, 1024], F32, tag="rd", name="rd")
                        nc.gpsimd.dma_start(out=rd, in_=cp[DH:VW, :])
                        rd_b = bass.AP(
                            tensor=rd.tensor, offset=rd.offset,
                            ap=[[0, 64], rd.ap[-1]],
                        )
                        L = lbp.tile([P, 1024], F32, tag="L", bufs=2)
                        nc.gpsimd.dma_start(
                            out=L[h * 64:(h + 1) * 64, :], in_=rd_b
                        )
                        if h == 0:
                            nc.vector.tensor_mul(
                                ot[0:64, q0:q0 + 1024],
                                cp[0:DH, :], L[0:64, :],
                            )
                        else:
                            otf = lbp.tile([P, 1024], F32, tag="otf",
                                           bufs=2)
                            nc.gpsimd.dma_start(
                                out=otf[64:128, :], in_=cp[0:DH, :]
                            )
                            nc.vector.tensor_mul(
                                ot[64:128, q0:q0 + 1024],
                                otf[64:128, :], L[64:128, :],
                            )

            # ---------- schedule -----------------------------------------
            load_batch(0)
            load_batch(1)
            for f in kproj_thunks(0):
                f()
            for f in vproj_thunks(0):
                f()
            for f in qproj_thunks(0):
                f()
            for b in range(B):
                if b + 2 < B:
                    load_batch(b + 2)
                if b + 1 < B:
                    fills.extend(kproj_thunks(b + 1))
                    fills.extend(vproj_thunks(b + 1))
                    fills.extend(qproj_thunks(b + 1))
                attention(b)
                fills.extend(oproj_thunks(b))
            drain(10 ** 9)

    nc.compile()
    return nc


def _prepare(x_Q, x_K, x_V, src_batch_lens, Wq, bq, Wk, bk, Wv, bv, Wo, bo):
    x_Q = np.asarray(x_Q, dtype=np.float32)
    x_K = np.asarray(x_K, dtype=np.float32)
    x_V = np.asarray(x_V, dtype=np.float32)
    lens = np.asarray(src_batch_lens)
    Wq = np.asarray(Wq, dtype=np.float32)
    Wk = np.asarray(Wk, dtype=np.float32)
    Wv = np.asarray(Wv, dtype=np.float32)
    Wo = np.asarray(Wo, dtype=np.float32)
    bq = np.asarray(bq, dtype=np.float32)
    bv = np.asarray(bv, dtype=np.float32)
    bo = np.asarray(bo, dtype=np.float32)

    kcs = []
    for b in range(B):
        lb = max(1, min(S, int(lens[b])))
        kcs.append((lb + P - 1) // P)
    klens = [k * P for k in kcs]

    # host-folded bias for the summed output
    bo2_full = (bv @ Wo + bo).astype(np.float32)      # [D]

    # full-query transposed activations (shared across cores)
    xqT = np.ascontiguousarray(
        x_Q.reshape(BS, D).T.astype(BF))              # [D, 8192]
    xkT = np.ascontiguousarray(np.concatenate(
        [x_K[b, 0:klens[b], :] for b in range(B)], axis=0).T.astype(BF))
    xvT = np.ascontiguousarray(np.concatenate(
        [x_V[b, 0:klens[b], :] for b in range(B)], axis=0).T.astype(BF))

    mcols = []
    for b in range(B):
        k_idx = np.arange(klens[b])
        mvec = np.where(k_idx < int(lens[b]), 0.0,
                        MASK_NEG).astype(np.float32)
        mcols.append(mvec.reshape(kcs[b], P).T)       # [128, kc_b]
    maskb = np.ascontiguousarray(np.concatenate(mcols, axis=1))

    in_maps = []
    for c in range(8):
        h0 = 2 * c * DH                                # head-slice offset
        in_maps.append({
            "xqT": xqT, "xkT": xkT, "xvT": xvT,
            "wq2": np.ascontiguousarray(Wq[:, h0:h0 + P].astype(BF)),
            "wk2": np.ascontiguousarray(Wk[:, h0:h0 + P].astype(BF)),
            "wv2": np.ascontiguousarray(Wv[:, h0:h0 + P].astype(BF)),
            "wo2": np.ascontiguousarray(Wo[h0:h0 + P, :].astype(BF)),
            "bq2": np.ascontiguousarray(
                (bq[h0:h0 + P] / 8.0).reshape(P, 1)),
            "maskb": maskb,
        })
    return tuple(kcs), bo2_full, in_maps


def _build_in_maps(inputs):
    return _prepare(**inputs)[2]


def kernel(x_Q, x_K, x_V, src_batch_lens, Wq, bq, Wk, bk, Wv, bv, Wo, bo):
    kcs, bo2_full, in_maps = _prepare(x_Q, x_K, x_V, src_batch_lens,
                                      Wq, bq, Wk, bk, Wv, bv, Wo, bo)
    if kcs not in _CACHE:
        _CACHE[kcs] = build_bass(kcs)
    nc = _CACHE[kcs]

    res = run_bass_kernel_spmd(nc, in_maps, core_ids=list(range(8)))

    acc = np.zeros((D, BS), dtype=np.float32)
    for c in range(8):
        acc += np.asarray(res.results[c]["yT"]).astype(np.float32)
    out = acc.T.reshape(B, S, D) + bo2_full
    return np.ascontiguousarray(out)
